# revision 53
# baseline (speedup 1.0000x reference)
import math
import os
import sys

import numpy as np

# nn_AxialAttentionD: B,C,D,H,W = 1,64,48,64,128; 4 heads, head_dim 16.
# Attention over D independently per (head, h, w). Sharded over H across
# 8 NeuronCores (8 H-rows per core). Bass/Tile kernel; per-core pipeline
# (chunk = 64 spatial positions = half an H-row; on-chip tensors are
# w-major: col = w*48 + d, so per-position slices are contiguous):
#   1. DMA x chunk [64, (d,w)] fp32 (d-major: DMA-friendly 256B runs)
#   2. QKV GEMMs, rhs AP streams x in w-major order; f32r for full PE
#      rate; padded q/k (4h x 32 rows: 16 data + 16 zeros via zero
#      weight cols) evicted bf16 with fused +pe (DVE tensor_tensor)
#   3. v: per-position PE transpose [64,48]->[48,64] into a psum bank
#      (8 positions), scatter-evict into vv [48, (w, 4h x (16v|16ones))]
#   4. per position: 4 row-tiled QK matmuls (K=32 head blocks, zeros
#      mask the other head halves) -> T^T = k~^T q~ in psum [112, 384]
#   5. batched exp on ACT (scale 1/4) -> bf16
#   6. per position: 4 col-tiled AV matmuls, lhsT = [v_h | ones] ->
#      psum rows 32h..32h+16 = O_h^T, rows 32h+16.. = Z_h replicated
#   7. normalize: psum[0:112] / psum[16:128] (PSUM APs are exempt from
#      the 0/32/64/96 partition-base rule) -> opad bf16, w-major
#   8. proj GEMM, rhs AP streams opad in d-major; zero-padded weights
#      kill the Z/junk rows -> y fp32 d-major -> DMA out
# The harness calls kernel() below.

sys.path.insert(0, "/opt/trn_rl_repo")

NUM_HEADS = 4
C = 64
D = 48
H = 64
W = 128
DIM = 16
N_CORES = 8
HSH = H // N_CORES          # H rows per core shard
W_C = 64                    # positions per chunk (half an H row)
N_C = D * W_C               # cols per chunk (3072)
EXP_GROUP = 4               # positions per QK/exp/AV/norm group (1 psum bank)
VT_GROUP = 8                # positions per v-transpose psum bank (8*64=512)


def _sinusoidal_pe(dim: int, depth: int) -> np.ndarray:
    half = (dim + 1) // 2
    inv_freq = np.exp(
        np.arange(half, dtype=np.float32) * (-math.log(10000.0) / max(1, half - 1))
    )
    pos = np.arange(depth, dtype=np.float32)
    angles = pos[:, None] * inv_freq[None, :]
    sin = np.sin(angles).T.astype(np.float32)
    cos = np.cos(angles).T.astype(np.float32)
    pe = np.zeros((dim, depth), dtype=np.float32)
    even = dim // 2
    if even > 0:
        pe[0 : 2 * even : 2, :] = sin[:even]
        pe[1 : 2 * even : 2, :] = cos[:even]
    if dim % 2 == 1:
        pe[-1, :] = sin[-1]
    return pe


def _prep_weights(qkv_w: np.ndarray, proj_w: np.ndarray):
    """Host-side: pad + transpose weights for the device layouts."""
    wq = qkv_w[0:C, :]            # [64, 64] rows (h,cc)
    wk = qkv_w[C : 2 * C, :]
    wv = qkv_w[2 * C : 3 * C, :]
    wqT_pad = np.zeros((C, 128), dtype=np.float32)
    wkT_pad = np.zeros((C, 128), dtype=np.float32)
    for h in range(NUM_HEADS):
        wqT_pad[:, 32 * h : 32 * h + 16] = wq[h * 16 : h * 16 + 16, :].T
        wkT_pad[:, 32 * h : 32 * h + 16] = wk[h * 16 : h * 16 + 16, :].T
    wvT = np.ascontiguousarray(wv.T)  # [64, 64] lhsT for v pass
    wpT_pad = np.zeros((128, C), dtype=np.float32)
    for h in range(NUM_HEADS):
        wpT_pad[32 * h : 32 * h + 16, :] = proj_w[:, h * 16 : h * 16 + 16].T
    # pe tile in w-major chunk layout [128 pad rows, (w, d)]
    pe = _sinusoidal_pe(DIM, D)  # [16, 48]
    pe_pad = np.zeros((128, D), dtype=np.float32)
    for h in range(NUM_HEADS):
        pe_pad[32 * h : 32 * h + 16, :] = pe
    pe_t = np.tile(pe_pad, (1, W_C))  # [128, N_C] w-major
    return wqT_pad, wkT_pad, wvT, wpT_pad, pe_t


def build_bass(hsh: int = HSH, sim_pad: bool = False, stage: str = "full"):
    import concourse.bacc as bacc
    import concourse.bass as bass
    import concourse.mybir as mybir
    from concourse import masks, tile

    f32 = mybir.dt.float32
    f32r = mybir.dt.float32r
    bf16 = mybir.dt.bfloat16

    nc = bacc.Bacc("TRN2", target_bir_lowering=False, debug=False)

    x_in = nc.dram_tensor("x", [C, D, hsh, W], f32, kind="ExternalInput")
    wq_d = nc.dram_tensor("wqT_pad", [C, 128], f32, kind="ExternalInput")
    wk_d = nc.dram_tensor("wkT_pad", [C, 128], f32, kind="ExternalInput")
    wv_d = nc.dram_tensor("wvT", [C, C], f32, kind="ExternalInput")
    wp_d = nc.dram_tensor("wpT_pad", [128, C], f32, kind="ExternalInput")
    pe_d = nc.dram_tensor("pe_t", [128, N_C], f32, kind="ExternalInput")
    y_out = nc.dram_tensor("y", [C, D, hsh, W], f32, kind="ExternalOutput")

    n_chunks = hsh * (W // W_C)
    # QKV GEMM slices: 8 positions x 48 = 384 cols (position-aligned)
    QKV_SL = 8 * D
    n_qkv = W_C // 8
    # proj slices: 8 d-rows x 64 w = 512 cols (d-major output)
    n_proj = D // 8

    with tile.TileContext(nc) as tc:
        with (
            tc.tile_pool(name="const", bufs=1) as constp,
            tc.tile_pool(name="xin", bufs=2) as xp,
            tc.tile_pool(name="qk", bufs=2) as qkp,
            tc.tile_pool(name="vd", bufs=2) as vp,
            tc.tile_pool(name="vv", bufs=2) as vvp,
            tc.tile_pool(name="texp", bufs=3) as texpp,
            tc.tile_pool(name="opad", bufs=2) as opadp,
            tc.tile_pool(name="yo", bufs=2) as yop,
            tc.tile_pool(name="gemm_ps", bufs=2, space="PSUM") as gpsp,
            tc.tile_pool(name="t_ps", bufs=1, space="PSUM") as tpsp,
            tc.tile_pool(name="o_ps", bufs=2, space="PSUM") as opsp,
        ):
            # ---- constants ----
            wq_f = constp.tile([C, 128], f32, tag="wq_f")
            wk_f = constp.tile([C, 128], f32, tag="wk_f")
            wv_f = constp.tile([C, C], f32, tag="wv_f")
            wp_f = constp.tile([128, C], f32, tag="wp_f")
            pe_sb = constp.tile([128, N_C], f32, tag="pe")
            nc.sync.dma_start(wq_f[:], wq_d.ap())
            nc.sync.dma_start(wk_f[:], wk_d.ap())
            nc.sync.dma_start(wv_f[:], wv_d.ap())
            nc.sync.dma_start(wp_f[:], wp_d.ap())
            nc.sync.dma_start(pe_sb[:], pe_d.ap())
            wp_b = constp.tile([128, C], bf16, tag="wp_b")
            nc.vector.tensor_copy(wp_b[:], wp_f[:])
            wq_b = constp.tile([C, 128], bf16, tag="wq_b")
            nc.vector.tensor_copy(wq_b[:], wq_f[:])
            wk_b = constp.tile([C, 128], bf16, tag="wk_b")
            nc.vector.tensor_copy(wk_b[:], wk_f[:])
            wv_b = constp.tile([C, C], bf16, tag="wv_b")
            nc.vector.tensor_copy(wv_b[:], wv_f[:])
            ident = constp.tile([C, C], bf16, tag="ident")
            masks.make_identity(nc, ident[:])
            ones32 = constp.tile([112, 32], bf16, tag="ones32")
            nc.vector.memset(ones32[:], 1.0)

            # persistent double-buffered tiles with constant regions
            vv_tiles = []
            opad_tiles = []
            for i in range(2):
                # vv: [112 (d=j, even pos rows 0-47 / odd rows 64-111),
                #      (w-pair, 4h x (16 v | 16 ones))]
                vv = vvp.tile([112, (W_C // 2) * 128], bf16, tag=f"vv{i}")
                vvr = vv[:].rearrange("p (w c) -> p w c", c=128)
                for h in range(NUM_HEADS):
                    nc.vector.memset(vvr[:, :, 32 * h + 16 : 32 * h + 32], 1.0)
                vv_tiles.append(vv)
                op = opadp.tile([128, N_C], bf16, tag=f"opad{i}")
                opad_tiles.append(op)

            # 4 persistent per-head psum banks: matmuls with different
            # tile_position ROWS must not share a psum bank on this HW, so
            # head h's T matmuls (row 32h) get their own bank. Rows 48-63
            # are never matmul-written but are read by the batched exp —
            # zero them once (data persists across reuse). Banks 0/1 also
            # double (via bf16 views) as the v-transpose scratch.
            t_banks = []
            for i in range(4):
                # bank 0 is a full 2KB bank: cols 384-511 (f32 view) hold
                # the v-transpose scratch (row config 0, same as head 0)
                tp = tpsp.tile([112, 512 if i == 0 else 8 * D], f32, tag=f"t{i}")
                nc.vector.memset(tp[32:64, :], 0.0)
                t_banks.append(tp)

            for ci in range(n_chunks):
                hs = ci // (W // W_C)
                wb = ci % (W // W_C)
                vv = vv_tiles[ci % 2]
                vvr = vv[:].rearrange("p (w c) -> p w c", c=128)
                opad = opad_tiles[ci % 2]

                # ---- 1. DMA x chunk (d-major storage) ----
                x_t = xp.tile([C, D, W_C], f32, tag="x")
                nc.sync.dma_start(
                    x_t[:], x_in.ap()[:, :, hs, wb * W_C : (wb + 1) * W_C]
                )
                # cast x to bf16; w-major streaming view [64, w, d]
                x_b = xp.tile([C, D, W_C], bf16, tag="xb")
                nc.vector.tensor_copy(x_b[:], x_t[:])
                x_wm = x_b[:].rearrange("p d w -> p w d")

                # ---- 2. QKV GEMMs (w-major outputs) ----
                q_t = qkp.tile([128, N_C], bf16, tag="q")
                k_t = qkp.tile([128, N_C], bf16, tag="k")
                v_t = vp.tile([C, N_C], bf16, tag="v")
                for si in range(n_qkv):
                    sl = slice(si * QKV_SL, (si + 1) * QKV_SL)
                    rhs = x_wm[:, si * 8 : (si + 1) * 8, :]
                    ps_q = gpsp.tile([128, QKV_SL], f32, tag="gemm")
                    nc.tensor.matmul(ps_q[:], wq_b[:], rhs, start=True, stop=True)
                    nc.vector.tensor_tensor(
                        q_t[:, sl], ps_q[:], pe_sb[:, sl], mybir.AluOpType.add
                    )
                    ps_k = gpsp.tile([128, QKV_SL], f32, tag="gemm")
                    nc.tensor.matmul(ps_k[:], wk_b[:], rhs, start=True, stop=True)
                    nc.vector.tensor_tensor(
                        k_t[:, sl], ps_k[:], pe_sb[:, sl], mybir.AluOpType.add
                    )
                    ps_v = gpsp.tile([128, QKV_SL], f32, tag="gemm")
                    nc.tensor.matmul(
                        ps_v[0:C, :], wv_b[:], rhs, start=True, stop=True
                    )
                    nc.vector.tensor_copy(v_t[:, sl], ps_v[0:C, :])

                qr = q_t[:].rearrange("p (w d) -> p w d", d=D)
                kr = k_t[:].rearrange("p (w d) -> p w d", d=D)
                vr = v_t[:].rearrange("p (w d) -> p w d", d=D)

                if stage == "qkv":
                    y_sb = yop.tile([C, N_C], f32, tag="y")
                    nc.vector.tensor_copy(y_sb[:], q_t[0:C, :])
                    nc.sync.dma_start(
                        y_out.ap()[:, :, hs, wb * W_C : (wb + 1) * W_C],
                        y_sb[:].rearrange("p (d w) -> p d w", w=W_C),
                    )
                    continue

                # ---- 3. v transpose into vv ----
                # 8 positions (4 w-pairs) per bank view: even pos -> rows
                # 0-47, odd -> rows 64-111, pair slot = 64 cols
                for vg in range(W_C // VT_GROUP):
                    vt_ps = t_banks[0][:, 384:512].bitcast(bf16)
                    for pi in range(VT_GROUP):
                        p = vg * VT_GROUP + pi
                        rbase = 64 * (pi % 2)
                        s = pi // 2
                        nc.tensor.transpose(
                            vt_ps[rbase : rbase + D, s * C : (s + 1) * C],
                            vr[:, p, :],
                            ident[:],
                            tile_position=(0, rbase),
                        )
                    # scatter-evict into the v sub-columns of vv
                    vt_r = vt_ps.rearrange(
                        "p (g h c) -> p g h c", h=NUM_HEADS, c=DIM
                    )
                    npair = VT_GROUP // 2
                    dst = vvr[
                        :, vg * npair : (vg + 1) * npair, :
                    ].rearrange("p g (h c) -> p g h c", h=NUM_HEADS)
                    nc.vector.tensor_copy(
                        dst[:, :, :, 0:DIM], vt_r[:, :, :, :]
                    )

                if stage == "vt":
                    y_sb = yop.tile([C, N_C], f32, tag="y")
                    nc.vector.tensor_copy(y_sb[:], vv[0:C, 0:N_C])
                    nc.sync.dma_start(
                        y_out.ap()[:, :, hs, wb * W_C : (wb + 1) * W_C],
                        y_sb[:].rearrange("p (d w) -> p d w", w=W_C),
                    )
                    continue

                # ---- 4-7. attention, 16 positions (8 w-pairs) at a time ----
                # T matmuls for head h go to t_banks[h] (uniform row config
                # per bank); bank cols = pair slot * 48, rows = parity.
                AG = 16
                for g in range(W_C // AG):
                    for pi in range(AG):
                        p = g * AG + pi
                        rbase = 64 * (pi % 2)
                        cbase = (pi // 2) * D
                        for h in range(NUM_HEADS):
                            nc.tensor.matmul(
                                t_banks[h][
                                    rbase : rbase + D, cbase : cbase + D
                                ],
                                kr[32 * h : 32 * h + 32, p, :],
                                qr[32 * h : 32 * h + 32, p, :],
                                start=True,
                                stop=True,
                                tile_position=(32 * h, rbase),
                            )
                    te = texpp.tile([112, 4 * 8 * D], bf16, tag="te")
                    for h in range(NUM_HEADS):
                        nc.scalar.activation(
                            te[:, h * 8 * D : (h + 1) * 8 * D],
                            t_banks[h][:, 0 : 8 * D],
                            mybir.ActivationFunctionType.Exp,
                            scale=0.25,
                        )
                    # AV + Z replicas: parity-split psum banks (a bank must
                    # keep a single tile_position row config) — per 8
                    # positions: even bank (rows 0-47 of te/vv) and odd
                    # bank (rows 64-111), 4 positions x (O | Z) each.
                    zoff = 4 * D
                    opr = opad[:].rearrange("p (w d) -> p w d", d=D)
                    for sg in range(AG // 8):
                        for par in range(2):
                            rbase = 64 * par
                            o_ps = opsp.tile([128, 8 * D], f32, tag="o")
                            for pi in range(4):
                                pl = sg * 8 + 2 * pi + par
                                p = g * AG + pl
                                for h in range(NUM_HEADS):
                                    tes = te[
                                        rbase : rbase + D,
                                        h * 8 * D
                                        + (pl // 2) * D : h * 8 * D
                                        + (pl // 2) * D
                                        + D,
                                    ]
                                    nc.tensor.matmul(
                                        o_ps[
                                            32 * h : 32 * h + 32,
                                            48 * pi : 48 * pi + 48,
                                        ],
                                        vvr[
                                            rbase : rbase + D,
                                            p // 2,
                                            32 * h : 32 * h + 32,
                                        ],
                                        tes,
                                        start=True,
                                        stop=True,
                                        tile_position=(rbase, 32 * h),
                                    )
                                    nc.tensor.matmul(
                                        o_ps[
                                            32 * h : 32 * h + 32,
                                            zoff + 48 * pi : zoff + 48 * pi + 48,
                                        ],
                                        ones32[rbase : rbase + D, :],
                                        tes,
                                        start=True,
                                        stop=True,
                                        tile_position=(rbase, 32 * h),
                                    )
                            # normalize: O * (1/Z); TT reads one PSUM operand
                            zr = texpp.tile([128, 4 * D], f32, tag="zr")
                            nc.vector.reciprocal(zr[:], o_ps[:, zoff : 2 * zoff])
                            w0 = g * AG + sg * 8 + par
                            nc.vector.tensor_tensor(
                                opr[:, w0 : w0 + 7 : 2, :],
                                o_ps[:, 0:zoff].rearrange(
                                    "p (g d) -> p g d", d=D
                                ),
                                zr[:].rearrange("p (g d) -> p g d", d=D),
                                mybir.AluOpType.mult,
                            )

                # ---- 8. proj (rhs streamed d-major) + out ----
                y_sb = yop.tile([C, N_C], f32, tag="y")
                o_dm = opad[:].rearrange("p (w d) -> p d w", d=D)
                for si in range(n_proj):
                    sl = slice(si * 8 * W_C, (si + 1) * 8 * W_C)
                    ps_y = gpsp.tile([128, 8 * W_C], f32, tag="gemm")
                    nc.tensor.matmul(
                        ps_y[0:C, :],
                        wp_b[:],
                        o_dm[:, si * 8 : (si + 1) * 8, :],
                        start=True,
                        stop=True,
                    )
                    nc.vector.tensor_copy(y_sb[:, sl], ps_y[0:C, :])
                nc.sync.dma_start(
                    y_out.ap()[:, :, hs, wb * W_C : (wb + 1) * W_C],
                    y_sb[:].rearrange("p (d w) -> p d w", w=W_C),
                )

    nc.compile()
    return nc


def _run_hw(x: np.ndarray, qkv_w: np.ndarray, proj_w: np.ndarray) -> np.ndarray:
    from concourse.bass_utils import run_bass_kernel_spmd

    B = x.shape[0]
    assert x.shape == (B, C, D, H, W)
    wqT_pad, wkT_pad, wvT, wpT_pad, pe_t = _prep_weights(qkv_w, proj_w)
    nc = build_bass(HSH)

    out = np.empty_like(x)
    for b in range(B):
        in_maps = []
        for core in range(N_CORES):
            xs = np.ascontiguousarray(
                x[b, :, :, core * HSH : (core + 1) * HSH, :]
            )
            in_maps.append(
                {
                    "x": xs,
                    "wqT_pad": wqT_pad,
                    "wkT_pad": wkT_pad,
                    "wvT": wvT,
                    "wpT_pad": wpT_pad,
                    "pe_t": pe_t,
                }
            )
        res = run_bass_kernel_spmd(nc, in_maps, list(range(N_CORES)))
        for core in range(N_CORES):
            out[b, :, :, core * HSH : (core + 1) * HSH, :] = res.results[core]["y"]
    return out


def kernel(x: np.ndarray, qkv_w: np.ndarray, proj_w: np.ndarray) -> np.ndarray:
    x = np.asarray(x, dtype=np.float32)
    qkv_w = np.asarray(qkv_w, dtype=np.float32)
    proj_w = np.asarray(proj_w, dtype=np.float32)
    return _run_hw(x, qkv_w, proj_w)
